# revision 1
# baseline (speedup 1.0000x reference)
"""AdaptiveGraphConv (Chebyshev K=3 graph conv) on 8 TRN2 NeuronCores.

Row-sharded over the 4096 nodes: core k owns nodes [512k, 512(k+1)).

Math (S = diag(s), s = d^-1/2 masked, A binary adj, L = I - S A S):
  out = h(W0-W2) + (Lh)W1 + 2 L(L h) W2 + bias
      = P0 + M - S Z3;  M = P1 + 2 P2 - 2 S Z2,
  Z2 = A(S P2), Z3 = A(S M), P0 = h(W0-W2), Pj = h Wj.

Key design points (see NOTES.md for the optimization log):
 - Host staging: x shipped bf16 in (b,c)-major/(t,n) layout; adj column-slice
   shipped fp8-e4m3 (EXACT for a binary matrix, 4x less HBM than f32; mixed
   fp8-lhsT x bf16-rhs matmul works on TRN2); block-diag weight concat
   [W1|W2|W0-W2] and bias replica prebuilt on host.
 - Degrees without a collective: d[shard] = column sums of the local adj
   slice (= row sums by symmetry) via PE matmul accumulation against ones.
 - Channel mixes computed as x_block^T @ W_cat (x stationary): one matmul
   per (mj, t) block lands all three mixes node-major -> no transposes.
 - The gathered operand is split into 3 F-column chunks; each pass runs
   3 x (AllGather chunk -> 4-bank matmul round -> epilogue) with the next
   chunk's collective in flight (the CC stream serializes collectives at
   ~30-40us each and is the pacing resource).
 - p1n (f32) holds P1 -> M -> out_n in place; epilogues are per-partition
   scalar_tensor_tensor ops; exit (PE transpose back + bias on ScalarE +
   block DMA) is fused into the MM2 epilogue per chunk.
"""

from contextlib import ExitStack

import ml_dtypes
import numpy as np

import concourse.bacc as bacc
import concourse.mybir as mybir
import concourse.tile as tile
from concourse.bass_utils import run_bass_kernel_spmd
from concourse.masks import make_identity

P = 128
NCORES = 8
N = 4096
S = N // NCORES          # 512 nodes per core
B, C, T = 4, 32, 12
F = B * C * T            # 1536 flattened (t, bo) columns: f = 128*t + 32*b + o
NT = S * T               # 6144 free columns in (b,c)-major (t, n) layout
KT = N // P              # 32 contraction tiles
MJ = S // P              # 4 node tiles per core; also AG chunk / phase count
FB = 512                 # matmul moving-free block
NFB = F // FB            # 3
KPP = KT // MJ           # 8 ki-tiles per phase

f32 = mybir.dt.float32
bf16 = mybir.dt.bfloat16
fp8 = mybir.dt.float8e4
ALU = mybir.AluOpType
ACT_FN = mybir.ActivationFunctionType

_CACHE = {}


def _graph_kernel(ctx, tc, xs, adjT, w, bias, out):
    nc = tc.nc
    RG = [list(range(NCORES))]

    consts = ctx.enter_context(tc.tile_pool(name="consts", bufs=1))
    persist = ctx.enter_context(tc.tile_pool(name="persist", bufs=1))
    scratch = ctx.enter_context(tc.tile_pool(name="scratch", bufs=10))
    stream = ctx.enter_context(tc.tile_pool(name="stream", bufs=4))
    psum = ctx.enter_context(tc.tile_pool(name="psum", bufs=1, space="PSUM"))
    dram = ctx.enter_context(tc.tile_pool(name="dram", bufs=1, space="DRAM"))

    # ---------------- constants (wcat/brep prebuilt on host)
    ones_bf = consts.tile([P, 1], fp8)
    nc.vector.memset(ones_bf[:], 1.0)
    wcat = consts.tile([P, 3 * P], bf16)
    nc.sync.dma_start(wcat[:], w[:])
    brep = consts.tile([P, 1], f32)
    nc.sync.dma_start(brep[:], bias[:])
    ident = consts.tile([P, P], f32)
    make_identity(nc, ident[:])

    # ---------------- node-major state: [p, mj, f], n_local = 128*mj + p,
    # f = 128*t + bo
    p1n = persist.tile([P, MJ, F], f32)       # P1 -> M -> out_n in place
    pX = persist.tile([P, MJ, T, 2 * P], bf16)  # [P2 | P0] per (mj, t) block
    ustage = persist.tile([P, MJ, F], bf16)   # AG staging (scaled bf16)
    p1n_v = p1n.rearrange("p m (t o) -> p m t o", t=T)
    ustage_v = ustage.rearrange("p m (t o) -> p m t o", t=T)

    # ---------------- entry mixes + adjacency load, interleaved so the PE
    # alternates between tiny mix matmuls and degree accumulation while both
    # DMA streams flow.
    abf = persist.tile([P, KT, S], fp8)       # lhsT tiles, resident all kernel
    pd = psum.tile([1, S], f32, tag="pe", bufs=4, name="pd")
    AK = 4   # ki-tiles per adjacency DMA (bf16, straight into abf)
    xv = xs.rearrange("p (t n) -> p t n", t=T)
    adjv = adjT.rearrange("(g k p) m -> g p k m", k=AK, p=P)
    NG = KT // AK  # 8 adjacency chunks
    for g in range(NG):
        nc.sync.dma_start(abf[:, AK * g:AK * (g + 1), :], adjv[g])
        for k in range(AK):
            ki = AK * g + k
            nc.tensor.matmul(pd[:], ones_bf[:], abf[:, ki, :],
                             start=(ki == 0), stop=(ki == KT - 1))
        # six entry blocks per adjacency chunk
        for bi in range(6 * g, min(6 * (g + 1), MJ * T)):
            mj, t = bi // T, bi % T
            if t == 0:
                xcb = stream.tile([P, T, P], bf16, tag="xcb", bufs=1,
                                  name=f"xcb{mj}")
                nc.sync.dma_start(xcb[:], xv[:, :, P * mj:P * (mj + 1)])
            psE = psum.tile([P, 3 * P], f32, tag="pe", bufs=4,
                            name=f"psE_{mj}_{t}")
            nc.tensor.matmul(psE[:], xcb[:, t, :], wcat[:], start=True,
                             stop=True)
            if mj < 2:
                nc.scalar.copy(pX[:, mj, t, :], psE[:, P:3 * P])
                nc.vector.tensor_copy(p1n_v[:, mj, t, :], psE[:, 0:P])
            else:
                nc.vector.tensor_copy(pX[:, mj, t, :], psE[:, P:3 * P])
                nc.scalar.copy(p1n_v[:, mj, t, :], psE[:, 0:P])

    # degree bounce + s chain (DVE mostly idle before this)
    d_row = consts.tile([1, S], f32)
    nc.vector.tensor_copy(d_row[:], pd[:])
    d_dram = dram.tile([MJ, P], f32, name="d_dram")
    nc.sync.dma_start(
        d_dram.rearrange("a p -> (a p)").rearrange("(o s) -> o s", o=1), d_row[:])
    s_raw = consts.tile([P, MJ], f32)
    nc.sync.dma_start(s_raw[:], d_dram.rearrange("a p -> p a"))
    s_dc = consts.tile([P, MJ], f32)
    nc.vector.tensor_scalar_max(s_dc[:], s_raw[:], 0.5)
    s_r = consts.tile([P, MJ], f32)
    nc.vector.reciprocal(s_r[:], s_dc[:])
    s_q = consts.tile([P, MJ], f32)
    nc.scalar.activation(s_q[:], s_r[:], ACT_FN.Sqrt)
    s_m = consts.tile([P, MJ], f32)
    nc.vector.tensor_scalar_min(s_m[:], s_raw[:], 1.0)
    s_t = consts.tile([P, MJ], f32)
    nc.vector.tensor_tensor(s_t[:], s_q[:], s_m[:], op=ALU.mult)
    sm2 = consts.tile([P, MJ], f32)   # -2s
    nc.vector.tensor_scalar_mul(sm2[:], s_t[:], -2.0)
    smn = consts.tile([P, MJ], f32)   # -s
    nc.vector.tensor_scalar_mul(smn[:], s_t[:], -1.0)

    # stage all of ustage (= P2 * s), then 3 AllGathers chunked by F columns.
    # The CC stream serializes collectives (~25-40us each), so keep it
    # continuously busy; matmuls pipeline one f-chunk behind it.
    for mj in range(MJ):
        nc.vector.tensor_scalar_mul(
            ustage_v[:, mj, :, :], pX[:, mj, :, 0:P], s_t[:, mj:mj + 1])
    ag1_out = [None] * NFB
    ag2_out = [None] * NFB
    for fi in range(NFB):
        fsl = slice(FB * fi, FB * (fi + 1))
        agi = dram.tile([MJ * P, FB], bf16, name=f"ag1i{fi}")
        ago = dram.tile([N, FB], bf16, addr_space="Shared", name=f"ag1o{fi}")
        nc.sync.dma_start(agi.rearrange("(m p) f -> p m f", p=P),
                          ustage[:, :, fsl])
        nc.gpsimd.collective_compute(
            "AllGather", ALU.bypass, replica_groups=RG,
            ins=[agi.opt()], outs=[ago.opt()],
        )
        ag1_out[fi] = ago

    TB = T // NFB

    def mm_pass(ag_bufs, tag, epilogue):
        # per f-chunk: rhs rows are plain global ki-tiles; 4 psum banks
        # (one per mj) accumulate over all 32 ki.
        for fi in range(NFB):
            uhq = []
            for q in range(MJ):
                uh = scratch.tile([P, KPP, FB], bf16, tag="sc",
                                  name=f"uh_{tag}_{fi}_{q}")
                nc.scalar.dma_start(
                    uh[:],
                    ag_bufs[fi].rearrange("(ki p) f -> p ki f", p=P)
                    [:, KPP * q:KPP * (q + 1), :])
                uhq.append(uh)
            pms = []
            for mj in range(MJ):
                pm = psum.tile([P, FB], f32, tag="pm", bufs=4,
                               name=f"pm_{tag}_{fi}_{mj}")
                for q in range(MJ):
                    for kk in range(KPP):
                        ki = KPP * q + kk
                        nc.tensor.matmul(
                            pm[:], abf[:, ki, P * mj:P * (mj + 1)],
                            uhq[q][:, kk, :],
                            start=(ki == 0), stop=(ki == KT - 1))
                pms.append(pm)
            epilogue(fi, pms)

    # ---------------- MM1: Z2 = A(s*P2); M = P1 + 2*P2 - 2*s*Z2 (in p1n)
    def epi1(fi, pms):
        fsl = slice(FB * fi, FB * (fi + 1))
        tsl = slice(TB * fi, TB * (fi + 1))
        for mj in range(MJ):
            nc.vector.scalar_tensor_tensor(
                p1n[:, mj, fsl], pms[mj][:], sm2[:, mj:mj + 1], p1n[:, mj, fsl],
                op0=ALU.mult, op1=ALU.add)
            nc.vector.scalar_tensor_tensor(
                p1n_v[:, mj, tsl, :], pX[:, mj, tsl, 0:P], 2.0,
                p1n_v[:, mj, tsl, :], op0=ALU.mult, op1=ALU.add)
            nc.vector.tensor_scalar_mul(
                ustage[:, mj, fsl], p1n[:, mj, fsl], s_t[:, mj:mj + 1])
        agi = dram.tile([MJ * P, FB], bf16, name=f"ag2i{fi}")
        ago = dram.tile([N, FB], bf16, addr_space="Shared", name=f"ag2o{fi}")
        nc.sync.dma_start(agi.rearrange("(m p) f -> p m f", p=P),
                          ustage[:, :, fsl])
        nc.gpsimd.collective_compute(
            "AllGather", ALU.bypass, replica_groups=RG,
            ins=[agi.opt()], outs=[ago.opt()],
        )
        ag2_out[fi] = ago

    mm_pass(ag1_out, "z2", epi1)

    # ---------------- MM2: Z3 = A(s*M); out_n = M - s*Z3 + P0; exit fused
    def epi2(fi, pms):
        fsl = slice(FB * fi, FB * (fi + 1))
        tsl = slice(TB * fi, TB * (fi + 1))
        for mj in range(MJ):
            nc.vector.scalar_tensor_tensor(
                p1n[:, mj, fsl], pms[mj][:], smn[:, mj:mj + 1], p1n[:, mj, fsl],
                op0=ALU.mult, op1=ALU.add)
            nc.gpsimd.tensor_tensor(
                p1n_v[:, mj, tsl, :], pX[:, mj, tsl, P:2 * P],
                p1n_v[:, mj, tsl, :], op=ALU.add)
        ov = out.rearrange("p (t n) -> p t n", t=T)
        for mj in range(MJ):
            pt = psum.tile([P, TB, P], f32, tag="pe", bufs=4,
                           name=f"pte_{fi}_{mj}")
            for j in range(TB):
                t = TB * fi + j
                nc.tensor.transpose(pt[:, j, :], p1n[:, mj, P * t:P * (t + 1)],
                                    ident[:])
            ob = stream.tile([P, TB, P], f32, tag="ob", bufs=2,
                             name=f"ob{fi}_{mj}")
            nc.scalar.activation(ob[:], pt[:], ACT_FN.Identity,
                                 bias=brep[:, 0:1])
            nc.sync.dma_start(
                ov[:, TB * fi:TB * (fi + 1), P * mj:P * (mj + 1)], ob[:])

    mm_pass(ag2_out, "z3", epi2)


def build_nc():
    nc = bacc.Bacc(target_bir_lowering=False)
    xs = nc.declare_dram_parameter("xs", [P, NT], bf16, isOutput=False)
    adjT = nc.declare_dram_parameter("adjT", [N, S], fp8, isOutput=False)
    w = nc.declare_dram_parameter("wcat", [P, 3 * P], bf16, isOutput=False)
    bias = nc.declare_dram_parameter("brep", [P, 1], f32, isOutput=False)
    out = nc.declare_dram_parameter("out", [P, NT], f32, isOutput=True)
    with tile.TileContext(nc) as tc, ExitStack() as ctx:
        _graph_kernel(ctx, tc, xs, adjT, w, bias, out)
    nc.compile()
    return nc


def make_in_maps(x, adj, weight, bias):
    wcat = np.zeros((P, 3 * P), np.float32)
    mats = [weight[1], weight[2], weight[0] - weight[2]]
    for j, m in enumerate(mats):
        for b in range(B):
            wcat[32 * b:32 * (b + 1), P * j + 32 * b:P * j + 32 * (b + 1)] = m
    wcat = wcat.astype(ml_dtypes.bfloat16)
    brep = np.tile(np.asarray(bias, np.float32), B).reshape(P, 1)
    in_maps = []
    for k in range(NCORES):
        sl = slice(S * k, S * (k + 1))
        xs = np.ascontiguousarray(
            x[:, :, sl, :].transpose(0, 1, 3, 2)).reshape(P, NT).astype(
                ml_dtypes.bfloat16)
        in_maps.append({
            "xs": xs,
            "adjT": np.ascontiguousarray(adj[:, sl]).astype(ml_dtypes.float8_e4m3),
            "wcat": wcat,
            "brep": brep,
        })
    return in_maps


def kernel(x, adj, weight, bias, _trace=False, _tmpdir=None):
    if "nc" not in _CACHE:
        _CACHE["nc"] = build_nc()
    nc = _CACHE["nc"]
    in_maps = make_in_maps(
        np.asarray(x, np.float32), np.asarray(adj, np.float32),
        np.asarray(weight, np.float32), np.asarray(bias, np.float32))
    res = run_bass_kernel_spmd(nc, in_maps, core_ids=list(range(NCORES)),
                               trace=_trace, tmpdir=_tmpdir)
    _CACHE["last_result"] = res
    parts = [r["out"].reshape(B, C, T, S).transpose(0, 1, 3, 2)
             for r in res.results]
    return np.concatenate(parts, axis=2)



# revision 3
# speedup vs baseline: 1.0404x; 1.0404x over previous
"""AdaptiveGraphConv (Chebyshev K=3 graph conv) on 8 TRN2 NeuronCores.

Data-parallel over the 48 (b, t) pairs: core k owns b = k//2, t in
[6*(k%2), 6*(k%2)+6) -> 192 local feature columns; the full 4096^2
adjacency is streamed fp8 to every core (binary matrix, exact in
fp8-e4m3), so there are NO large collectives at all.

Math (S = diag(s), s = d^-1/2 masked, G = S A S, L = I - G):
  out = P0 + M - S A (s*M);  M = P1 + P2' - 2 S A (s*P2'/2),
  P2' = h(2 W2), P1 = h W1, P0 = h(W0 - W2)   (channel mixes at entry).

Design:
 - Entry: x shipped bf16 in (pair,c)-major/node layout; per 128-node tile
   one matmul x_blk^T @ wcat (block-diag over 4 pair-slots) lands P1|P2'|P0
   node-major -> no transposes until exit.
 - Degrees without waiting for the full adjacency: a dedicated early
   [4096, 512] own-column slice gives local column sums (= row sums by
   symmetry) via PE accumulation against ones; a 16KB AllGather (~5us)
   distributes d to all cores. The CC stream is otherwise idle.
 - Both Laplacian applications are straight [4096 x 4096] x [4096 x 192]
   matmuls per core: adj streamed in 8 column chunks of 512 (double
   buffered, 6us DMA per chunk vs ~10us PE per chunk) in each pass.
 - Epilogues are per-mo-tile scalar_tensor_tensor ops split across
   DVE/GpSimd; exit (PE transpose back + bias on ScalarE + batched DMA)
   is fused into the pass-2 epilogue per chunk.
 - FP8_RHS switches the pass rhs (s*P2', s*M) to fp8-e4m3 and uses
   DoubleRow matmuls (2 k-rows/cycle).
"""

from contextlib import ExitStack

import ml_dtypes
import numpy as np

import concourse.bacc as bacc
import concourse.mybir as mybir
import concourse.tile as tile
from concourse.bass_utils import run_bass_kernel_spmd
from concourse.masks import make_identity

P = 128
NCORES = 8
N = 4096
SC = N // NCORES         # 512: own-column degree slice / adj chunk width
B, C, T = 4, 32, 12
NPAIR = 6                # (b, t) pairs per core
FL = NPAIR * C           # 192 local feature columns
KT = N // P              # 32 contraction tiles
NMO = N // P             # 32 output node tiles
NCHUNK = 8               # adjacency column chunks
MOC = NMO // NCHUNK      # 4 mo tiles per chunk

FP8_RHS = False

f32 = mybir.dt.float32
bf16 = mybir.dt.bfloat16
fp8 = mybir.dt.float8e4
ALU = mybir.AluOpType
ACT_FN = mybir.ActivationFunctionType
DR = mybir.MatmulPerfMode.DoubleRow
RHS_DT = fp8 if FP8_RHS else bf16

_CACHE = {}


def _graph_kernel(ctx, tc, xs0, xs1, adjd, adjb, w, bias, out):
    nc = tc.nc
    RG = [list(range(NCORES))]

    consts = ctx.enter_context(tc.tile_pool(name="consts", bufs=1))
    persist = ctx.enter_context(tc.tile_pool(name="persist", bufs=1))
    stream = ctx.enter_context(tc.tile_pool(name="stream", bufs=4))
    psum = ctx.enter_context(tc.tile_pool(name="psum", bufs=1, space="PSUM"))
    dram = ctx.enter_context(tc.tile_pool(name="dram", bufs=1, space="DRAM"))

    # ---------------- constants
    ones8 = consts.tile([P, 1], fp8)
    nc.vector.memset(ones8[:], 1.0)
    wcat = consts.tile([P, 3 * P], bf16)
    nc.scalar.dma_start(wcat[:], w[:])
    brep = consts.tile([P, 1], f32)
    nc.scalar.dma_start(brep[:], bias[:])
    ident = consts.tile([P, P], f32)
    make_identity(nc, ident[:])

    # ---------------- persistent node-major state [p, nt, f], n = 128*nt + p
    p1n = persist.tile([P, NMO, FL], f32)     # P1 -> M -> out_n in place
    pP2 = persist.tile([P, NMO, FL], bf16)    # P2' = h(2 W2)
    pP0 = persist.tile([P, NMO, FL], bf16)    # P0 = h(W0 - W2)
    ys = persist.tile([P, KT, FL], RHS_DT)    # pass-1 rhs: (s/2) * P2'
    uh2 = persist.tile([P, KT, FL], RHS_DT)   # pass-2 rhs: s * M
    xg0 = persist.tile([P, NMO, P], bf16)     # x^T pairs 0-3, (slot,c)-major
    xg1 = persist.tile([64, NMO, P], bf16)    # x^T pairs 4-5
    add = persist.tile([P, KT, SC], fp8)      # own-column degree slice

    # ---------------- entry: x DMA + degree slice DMA, channel-mix matmuls
    # and ones^T degree accumulation interleaved on the PE.
    nc.scalar.dma_start(xg0[:], xs0.rearrange("p (t n) -> p t n", t=NMO))
    nc.scalar.dma_start(xg1[:], xs1.rearrange("p (t n) -> p t n", t=NMO))
    addv = adjd.rearrange("(g k p) m -> g p k m", g=4, p=P)
    pd = psum.tile([1, SC], f32, tag="pd", bufs=1, name="pd")
    for g in range(4):
        nc.sync.dma_start(add[:, 8 * g:8 * (g + 1), :], addv[g])
        for kk in range(8):
            ki = 8 * g + kk
            nc.tensor.matmul(pd[:], ones8[:], add[:, ki, :],
                             start=(ki == 0), stop=(ki == KT - 1))
    for nt in range(NMO):
        psE0 = psum.tile([P, 3 * P], f32, tag="pe", bufs=3, name=f"e0_{nt}")
        nc.tensor.matmul(psE0[:], xg0[:, nt, :], wcat[:], start=True,
                         stop=True)
        psE1 = psum.tile([P, 3 * P], f32, tag="pe", bufs=3, name=f"e1_{nt}")
        nc.tensor.matmul(psE1[:], xg1[:, nt, :], wcat[0:64, :], start=True,
                         stop=True)
        nc.vector.tensor_copy(p1n[:, nt, 0:P], psE0[:, 0:P])
        nc.scalar.copy(p1n[:, nt, P:FL], psE1[:, 0:64])
        nc.scalar.copy(pP2[:, nt, 0:P], psE0[:, P:2 * P])
        nc.vector.tensor_copy(pP2[:, nt, P:FL], psE1[:, P:P + 64])
        nc.scalar.copy(pP0[:, nt, 0:P], psE0[:, 2 * P:3 * P])
        nc.vector.tensor_copy(pP0[:, nt, P:FL], psE1[:, 2 * P:2 * P + 64])

    # ---------------- degree AllGather (16KB) + s chain
    d_row = consts.tile([1, SC], f32)
    nc.vector.tensor_copy(d_row[:], pd[:])
    agdi = dram.tile([1, SC], f32, name="agdi")
    agdo = dram.tile([NCORES, SC], f32, addr_space="Shared", name="agdo")
    nc.sync.dma_start(agdi[:], d_row[:])
    nc.gpsimd.collective_compute(
        "AllGather", ALU.bypass, replica_groups=RG,
        ins=[agdi.opt()], outs=[agdo.opt()],
    )
    s_raw = consts.tile([P, NMO], f32)
    nc.scalar.dma_start(
        s_raw[:],
        agdo.rearrange("c m -> (c m)").rearrange("(t p) -> p t", p=P))
    s_dc = consts.tile([P, NMO], f32)
    nc.vector.tensor_scalar_max(s_dc[:], s_raw[:], 0.5)
    s_r = consts.tile([P, NMO], f32)
    nc.vector.reciprocal(s_r[:], s_dc[:])
    s_q = consts.tile([P, NMO], f32)
    nc.scalar.activation(s_q[:], s_r[:], ACT_FN.Sqrt)
    s_m = consts.tile([P, NMO], f32)
    nc.vector.tensor_scalar_min(s_m[:], s_raw[:], 1.0)
    s_t = consts.tile([P, NMO], f32)
    nc.vector.tensor_tensor(s_t[:], s_q[:], s_m[:], op=ALU.mult)
    s_h = consts.tile([P, NMO], f32)   # s/2 (ys scale: P2' = 2 P2)
    nc.vector.tensor_scalar_mul(s_h[:], s_t[:], 0.5)
    sm2 = consts.tile([P, NMO], f32)   # -2s
    nc.vector.tensor_scalar_mul(sm2[:], s_t[:], -2.0)
    smn = consts.tile([P, NMO], f32)   # -s
    nc.vector.tensor_scalar_mul(smn[:], s_t[:], -1.0)

    for nt in range(NMO):
        nc.vector.tensor_scalar_mul(ys[:, nt, :], pP2[:, nt, :],
                                    s_h[:, nt:nt + 1])

    # ---------------- the two Laplacian applications
    adjbv = adjb.rearrange("(j k p) m -> j p k m", j=NCHUNK, p=P)
    ov = out

    def mm_pass(rhs, tag, epilogue):
        if FP8_RHS:
            rv = rhs.rearrange("p (kp two) f -> p kp two f", two=2)
        for j in range(NCHUNK):
            ab = stream.tile([P, KT, SC], fp8, tag="ab", bufs=4,
                             name=f"ab_{tag}_{j}")
            nc.sync.dma_start(ab[:], adjbv[j])
            if FP8_RHS:
                abv = ab.rearrange("p (kp two) m -> p kp two m", two=2)
            for q in range(MOC):
                mo = MOC * j + q
                pm = psum.tile([P, FL], f32, tag="pm", bufs=4,
                               name=f"pm_{tag}_{mo}")
                if FP8_RHS:
                    for kp in range(KT // 2):
                        nc.tensor.matmul(
                            pm[:], abv[:, kp, :, P * q:P * (q + 1)],
                            rv[:, kp, :, :], start=(kp == 0),
                            stop=(kp == KT // 2 - 1), perf_mode=DR)
                else:
                    for ki in range(KT):
                        nc.tensor.matmul(
                            pm[:], ab[:, ki, P * q:P * (q + 1)],
                            rhs[:, ki, :], start=(ki == 0),
                            stop=(ki == KT - 1))
                epilogue(j, q, mo, pm)

    # ---------------- MM1: Z2 = A(s*P2); M = P1 + P2' - 2*s*Z2 (in p1n)
    def epi1(j, q, mo, pm):
        nc.vector.scalar_tensor_tensor(
            p1n[:, mo, :], pm[:], sm2[:, mo:mo + 1], p1n[:, mo, :],
            op0=ALU.mult, op1=ALU.add)
        nc.gpsimd.tensor_tensor(
            p1n[:, mo, :], pP2[:, mo, :], p1n[:, mo, :], op=ALU.add)
        nc.vector.tensor_scalar_mul(uh2[:, mo, :], p1n[:, mo, :],
                                    s_t[:, mo:mo + 1])

    mm_pass(ys, "z2", epi1)

    # ---------------- MM2: Z3 = A(s*M); out_n = M - s*Z3 + P0; fused exit
    ob0 = [None] * NCHUNK
    ob1 = [None] * NCHUNK

    def epi2(j, q, mo, pm):
        nc.vector.scalar_tensor_tensor(
            p1n[:, mo, :], pm[:], smn[:, mo:mo + 1], p1n[:, mo, :],
            op0=ALU.mult, op1=ALU.add)
        nc.gpsimd.tensor_tensor(
            p1n[:, mo, :], pP0[:, mo, :], p1n[:, mo, :], op=ALU.add)
        if q == 0:
            ob0[j] = stream.tile([P, MOC, P], f32, tag="ob0", bufs=2,
                                 name=f"ob0_{j}")
            ob1[j] = stream.tile([64, MOC, P], f32, tag="ob1", bufs=2,
                                 name=f"ob1_{j}")
        pt0 = psum.tile([P, P], f32, tag="pe", bufs=3, name=f"pt0_{mo}")
        nc.tensor.transpose(pt0[:], p1n[:, mo, 0:P], ident[:])
        nc.scalar.activation(ob0[j][:, q, :], pt0[:], ACT_FN.Identity,
                             bias=brep[:, 0:1])
        pt1 = psum.tile([64, P], f32, tag="pe", bufs=3, name=f"pt1_{mo}")
        nc.tensor.transpose(pt1[:], p1n[:, mo, P:FL], ident[:])
        nc.scalar.activation(ob1[j][:, q, :], pt1[:], ACT_FN.Identity,
                             bias=brep[0:64, 0:1])
        if q == MOC - 1:
            nc.scalar.dma_start(
                ov[0:P, SC * j:SC * (j + 1)],
                ob0[j].rearrange("p a b -> p (a b)"))
            nc.scalar.dma_start(
                ov[P:P + 64, SC * j:SC * (j + 1)],
                ob1[j].rearrange("p a b -> p (a b)"))

    mm_pass(uh2, "z3", epi2)


def build_nc():
    nc = bacc.Bacc(target_bir_lowering=False)
    xs0 = nc.declare_dram_parameter("xs0", [P, N], bf16, isOutput=False)
    xs1 = nc.declare_dram_parameter("xs1", [64, N], bf16, isOutput=False)
    adjd = nc.declare_dram_parameter("adjd", [N, SC], fp8, isOutput=False)
    adjb = nc.declare_dram_parameter("adjb", [NCHUNK * N, SC], fp8,
                                     isOutput=False)
    w = nc.declare_dram_parameter("wcat", [P, 3 * P], bf16, isOutput=False)
    bias = nc.declare_dram_parameter("brep", [P, 1], f32, isOutput=False)
    out = nc.declare_dram_parameter("out", [NPAIR * C, N], f32, isOutput=True)
    with tile.TileContext(nc) as tc, ExitStack() as ctx:
        _graph_kernel(ctx, tc, xs0, xs1, adjd, adjb, w, bias, out)
    nc.compile()
    return nc


def make_in_maps(x, adj, weight, bias):
    wcat = np.zeros((P, 3 * P), np.float32)
    mats = [weight[1], 2.0 * weight[2], weight[0] - weight[2]]
    for j, m in enumerate(mats):
        for s in range(4):
            wcat[32 * s:32 * (s + 1),
                 P * j + 32 * s:P * j + 32 * (s + 1)] = m
    wcat = wcat.astype(ml_dtypes.bfloat16)
    brep = np.tile(np.asarray(bias, np.float32), 4).reshape(P, 1)
    adj8 = np.ascontiguousarray(
        adj.reshape(N, NCHUNK, SC).transpose(1, 0, 2)).reshape(
            NCHUNK * N, SC).astype(ml_dtypes.float8_e4m3)
    in_maps = []
    for k in range(NCORES):
        b, t0 = k // 2, NPAIR * (k % 2)
        xk = x[b][:, :, t0:t0 + NPAIR].transpose(2, 0, 1)  # [pair, c, n]
        in_maps.append({
            "xs0": np.ascontiguousarray(xk[0:4]).reshape(P, N).astype(
                ml_dtypes.bfloat16),
            "xs1": np.ascontiguousarray(xk[4:6]).reshape(64, N).astype(
                ml_dtypes.bfloat16),
            "adjd": np.ascontiguousarray(
                adj[:, SC * k:SC * (k + 1)]).astype(ml_dtypes.float8_e4m3),
            "adjb": adj8,
            "wcat": wcat,
            "brep": brep,
        })
    return in_maps


def kernel(x, adj, weight, bias, _trace=False, _tmpdir=None):
    if "nc" not in _CACHE:
        _CACHE["nc"] = build_nc()
    nc = _CACHE["nc"]
    in_maps = make_in_maps(
        np.asarray(x, np.float32), np.asarray(adj, np.float32),
        np.asarray(weight, np.float32), np.asarray(bias, np.float32))
    res = run_bass_kernel_spmd(nc, in_maps, core_ids=list(range(NCORES)),
                               trace=_trace, tmpdir=_tmpdir)
    _CACHE["last_result"] = res
    full = np.empty((B, C, N, T), np.float32)
    for k, r in enumerate(res.results):
        b, t0 = k // 2, NPAIR * (k % 2)
        part = r["out"].reshape(NPAIR, C, N)
        full[b, :, :, t0:t0 + NPAIR] = part.transpose(1, 2, 0)
    return full


# revision 17
# speedup vs baseline: 1.2710x; 1.2216x over previous
"""AdaptiveGraphConv (Chebyshev K=3 graph conv) on 8 TRN2 NeuronCores.

Data-parallel over the 48 (b, t) pairs: core k owns b = k//2, t in
[6*(k%2), 6*(k%2)+6) -> 192 local feature columns; the full 4096^2
adjacency is streamed fp8 to every core (binary matrix, exact in
fp8-e4m3), so there are NO large collectives at all.

Math (S = diag(s), s = d^-1/2 masked, G = S A S, L = I - G):
  out = P0 + M - S A (s*M);  M = P1 + P2' - 2 S A (s*P2'/2),
  P2' = h(2 W2), P1 = h W1, P0 = h(W0 - W2) + bias (mixes at entry).

Schedule (the critical path to pass 1 is s and P2' only):
 - degrees first: an early [4096, 512] own-column slice gives local column
   sums (= row sums by symmetry) via fp8 DoubleRow accumulation against
   ones; a 16KB AllGather (~5us) distributes d. CC is otherwise idle.
 - entry loop 1 computes only the P2' mix (one 128-col matmul per node
   tile); the s chain and ys = (s/2)*P2' follow right behind in the DVE
   FIFO -> pass 1 starts ~20us in. Loop 2 (P1, P0+bias mixes) fills the
   PE gap while ys finishes; P1 drains on GpSimd, P0 on DVE.
 - each Laplacian pass streams adj in 8 column chunks of 512 (4-deep
   pool, 6us DMA vs ~10us PE per chunk); 4 psum banks accumulate.
 - exit is transpose-free: out is [node, f] on device; bias was folded
   into P0 at entry, so pass-2 epilogue DMAs p1n straight out. The host
   does the final [n, f] -> [b, c, n, t] transpose during reassembly.
 - FP8_RHS switches the pass rhs (s*P2', s*M) to fp8-e4m3 DoubleRow
   matmuls (2 k-rows per instruction).
"""

from contextlib import ExitStack

import ml_dtypes
import numpy as np

import concourse.bacc as bacc
import concourse.mybir as mybir
import concourse.tile as tile
from concourse.bass_utils import run_bass_kernel_spmd

P = 128
NCORES = 8
N = 4096
SC = N // NCORES         # 512: own-column degree slice / adj chunk width
B, C, T = 4, 32, 12
NPAIR = 6                # (b, t) pairs per core
FL = NPAIR * C           # 192 local feature columns
KT = N // P              # 32 contraction tiles
NMO = N // P             # 32 output node tiles
NCHUNK = 8               # adjacency column chunks
MOC = NMO // NCHUNK      # 4 mo tiles per chunk

FP8_RHS = False
DEBUG_DUMPS = False

f32 = mybir.dt.float32
bf16 = mybir.dt.bfloat16
fp8 = mybir.dt.float8e4
ALU = mybir.AluOpType
ACT_FN = mybir.ActivationFunctionType
DR = mybir.MatmulPerfMode.DoubleRow
RHS_DT = fp8 if FP8_RHS else bf16

_CACHE = {}


def _graph_kernel(ctx, tc, xs0, xs1, adjd, adjb, w, bfull_p, out, dumps=None):
    nc = tc.nc
    RG = [list(range(NCORES))]

    consts = ctx.enter_context(tc.tile_pool(name="consts", bufs=1))
    persist = ctx.enter_context(tc.tile_pool(name="persist", bufs=1))
    stream = ctx.enter_context(tc.tile_pool(name="stream", bufs=4))
    psum = ctx.enter_context(tc.tile_pool(name="psum", bufs=1, space="PSUM"))
    dram = ctx.enter_context(tc.tile_pool(name="dram", bufs=1, space="DRAM"))

    # ---------------- constants
    ones8 = consts.tile([P, 32], fp8)
    nc.vector.memset(ones8[:], 1.0)
    onesv = ones8.rearrange("p (two g) -> p two g", two=2)
    wcat = consts.tile([P, 3 * P], bf16)   # [P2' | P1 | P0]
    nc.scalar.dma_start(wcat[:], w[:])
    bfull = consts.tile([P, FL], bf16)
    nc.scalar.dma_start(bfull[:], bfull_p[:])

    # ---------------- persistent node-major state [p, nt, f], n = 128*nt + p
    p1n = persist.tile([P, NMO, FL], f32)     # P1 -> M -> out_n in place
    pP2 = persist.tile([P, NMO, FL], bf16)    # P2' = h(2 W2)
    pP0 = persist.tile([P, NMO, FL], bf16)    # P0 = h(W0 - W2) + bias
    ys = persist.tile([P, KT, FL], RHS_DT)    # pass-1 rhs: (s/2) * P2'
    uh2 = persist.tile([P, KT, FL], RHS_DT)   # pass-2 rhs: s * M
    xg0 = persist.tile([P, NMO, P], bf16)     # x^T pairs 0-3, (slot,c)-major
    xg1 = persist.tile([64, NMO, P], bf16)    # x^T pairs 4-5
    add = persist.tile([P, KT, SC], fp8)      # own-column degree slice
    addv = add.rearrange("p (kp two) m -> p kp two m", two=2)

    # x DMA in chunks so the first entry matmul can start ~2us in
    xv0 = xs0.rearrange("p (t n) -> p t n", t=NMO)
    xv1 = xs1.rearrange("p (t n) -> p t n", t=NMO)
    for c in range(4):
        nc.scalar.dma_start(xg0[:, 8 * c:8 * (c + 1), :],
                            xv0[:, 8 * c:8 * (c + 1), :])
        nc.scalar.dma_start(xg1[:, 8 * c:8 * (c + 1), :],
                            xv1[:, 8 * c:8 * (c + 1), :])

    # ---------------- degrees: own-column sums via fp8 DoubleRow vs ones
    adv = adjd.rearrange("(g k p) m -> g p k m", g=4, p=P)
    pd = psum.tile([1, SC], f32, tag="pd", bufs=1, name="pd")
    for g in range(4):
        nc.sync.dma_start(add[:, 8 * g:8 * (g + 1), :], adv[g])
        for kk in range(8):
            ki = 8 * g + kk
            nc.tensor.matmul(pd[:], ones8[:, 0:1], add[:, ki, :],
                             start=(ki == 0), stop=(ki == KT - 1))
    d_row = consts.tile([1, SC], f32)
    nc.vector.tensor_copy(d_row[:], pd[:])
    agdi = dram.tile([1, SC], f32, name="agdi")
    agdo = dram.tile([NCORES, SC], f32, addr_space="Shared", name="agdo")
    nc.sync.dma_start(agdi[:], d_row[:])
    nc.gpsimd.collective_compute(
        "AllGather", ALU.bypass, replica_groups=RG,
        ins=[agdi.opt()], outs=[agdo.opt()],
    )
    s_raw = consts.tile([P, NMO], f32)
    nc.scalar.dma_start(
        s_raw[:],
        agdo.rearrange("c m -> (c m)").rearrange("(t p) -> p t", p=P))

    # ---------------- entry loop 1: P2' mix only (critical path)
    for nt in range(NMO):
        psE0 = psum.tile([P, P], f32, tag="pe", bufs=3, name=f"e0_{nt}")
        nc.tensor.matmul(psE0[:], xg0[:, nt, :], wcat[:, 0:P], start=True,
                         stop=True)
        psE1 = psum.tile([P, P], f32, tag="pe", bufs=3, name=f"e1_{nt}")
        nc.tensor.matmul(psE1[:], xg1[:, nt, :], wcat[0:64, 0:P], start=True,
                         stop=True)
        nc.vector.tensor_copy(pP2[:, nt, 0:P], psE0[:])
        nc.vector.tensor_copy(pP2[:, nt, P:FL], psE1[:, 0:64])

    # s chain (after loop-1 drains in the DVE FIFO; sqrt alone on ScalarE)
    s_dc = consts.tile([P, NMO], f32)
    nc.vector.tensor_scalar_max(s_dc[:], s_raw[:], 0.5)
    s_r = consts.tile([P, NMO], f32)
    nc.vector.reciprocal(s_r[:], s_dc[:])
    s_q = consts.tile([P, NMO], f32)
    nc.scalar.activation(s_q[:], s_r[:], ACT_FN.Sqrt)
    s_m = consts.tile([P, NMO], f32)
    nc.vector.tensor_scalar_min(s_m[:], s_raw[:], 1.0)
    s_t = consts.tile([P, NMO], f32)
    nc.vector.tensor_tensor(s_t[:], s_q[:], s_m[:], op=ALU.mult)
    s_h = consts.tile([P, NMO], f32)   # s/2 (ys scale: P2' = 2 P2)
    nc.vector.tensor_scalar_mul(s_h[:], s_t[:], 0.5)
    sm2 = consts.tile([P, NMO], f32)   # -2s
    nc.vector.tensor_scalar_mul(sm2[:], s_t[:], -2.0)
    smn = consts.tile([P, NMO], f32)   # -s
    nc.vector.tensor_scalar_mul(smn[:], s_t[:], -1.0)
    for nt in range(NMO):
        nc.vector.tensor_scalar_mul(ys[:, nt, :], pP2[:, nt, :],
                                    s_h[:, nt:nt + 1])

    if dumps:
        nc.scalar.dma_start(dumps["dmp_d"][:], d_row[:])
        nc.scalar.dma_start(dumps["dmp_s"][:], s_t[:])
        nc.scalar.dma_start(dumps["dmp_ys"][:],
                            ys.rearrange("p a b -> p (a b)"))
        nc.scalar.dma_start(dumps["dmp_p2"][:],
                            pP2.rearrange("p a b -> p (a b)"))

    # ---------------- entry loop 2: P1 and P0+bias mixes (fills the PE gap
    # until ys is ready; P1 drains on GpSimd, P0 on DVE)
    for nt in range(NMO):
        psE2 = psum.tile([P, 2 * P], f32, tag="pe", bufs=3, name=f"f0_{nt}")
        nc.tensor.matmul(psE2[:], xg0[:, nt, :], wcat[:, P:3 * P], start=True,
                         stop=True)
        psE3 = psum.tile([P, 2 * P], f32, tag="pe", bufs=3, name=f"f1_{nt}")
        nc.tensor.matmul(psE3[:], xg1[:, nt, :], wcat[0:64, P:3 * P],
                         start=True, stop=True)
        nc.scalar.copy(p1n[:, nt, 0:P], psE2[:, 0:P])
        nc.scalar.copy(p1n[:, nt, P:FL], psE3[:, 0:64])
        nc.vector.tensor_tensor(pP0[:, nt, 0:P], psE2[:, P:2 * P],
                                bfull[:, 0:P], op=ALU.add)
        nc.vector.tensor_tensor(pP0[:, nt, P:FL], psE3[:, P:P + 64],
                                bfull[:, P:FL], op=ALU.add)

    if dumps:
        nc.scalar.dma_start(dumps["dmp_p1"][:],
                            p1n.rearrange("p a b -> p (a b)"))
        nc.scalar.dma_start(dumps["dmp_p0"][:],
                            pP0.rearrange("p a b -> p (a b)"))

    # ---------------- the two Laplacian applications
    abv = adjb.rearrange("(j k p) m -> j p k m", j=NCHUNK, p=P)
    outv = out.rearrange("(mo p) f -> p mo f", p=P)

    def mm_pass(rhs, tag, epilogue):
        rv = rhs.rearrange("p (kp two) f -> p kp two f", two=2)
        for j in range(NCHUNK):
            ab = stream.tile([P, KT, SC], fp8, tag="ab", bufs=4,
                             name=f"ab_{tag}_{j}")
            nc.sync.dma_start(ab[:], abv[j])
            abdr = ab.rearrange("p (kp two) m -> p kp two m", two=2)
            for q in range(MOC):
                mo = MOC * j + q
                pm = psum.tile([P, FL], f32, tag="pm", bufs=4,
                               name=f"pm_{tag}_{mo}")
                if FP8_RHS:
                    for kp in range(KT // 2):
                        nc.tensor.matmul(
                            pm[:], abdr[:, kp, :, P * q:P * (q + 1)],
                            rv[:, kp, :, :], start=(kp == 0),
                            stop=(kp == KT // 2 - 1), perf_mode=DR)
                else:
                    for ki in range(KT):
                        nc.tensor.matmul(
                            pm[:], ab[:, ki, P * q:P * (q + 1)],
                            rhs[:, ki, :], start=(ki == 0),
                            stop=(ki == KT - 1))
                epilogue(j, q, mo, pm)

    # ---------------- MM1: Z2 = A(s*P2); M = P1 + P2' - 2*s*Z2 (in p1n)
    def epi1(j, q, mo, pm):
        nc.vector.scalar_tensor_tensor(
            p1n[:, mo, :], pm[:], sm2[:, mo:mo + 1], p1n[:, mo, :],
            op0=ALU.mult, op1=ALU.add)
        nc.gpsimd.tensor_tensor(
            p1n[:, mo, :], pP2[:, mo, :], p1n[:, mo, :], op=ALU.add)
        nc.vector.tensor_scalar_mul(uh2[:, mo, :], p1n[:, mo, :],
                                    s_t[:, mo:mo + 1])

    mm_pass(ys, "z2", epi1)

    if dumps:
        nc.scalar.dma_start(dumps["dmp_m"][:],
                            p1n.rearrange("p a b -> p (a b)"))
        nc.scalar.dma_start(dumps["dmp_uh2"][:],
                            uh2.rearrange("p a b -> p (a b)"))

    # ---------------- MM2: Z3 = A(s*M); out_n = M - s*Z3 + P0; DMA out
    def epi2(j, q, mo, pm):
        nc.vector.scalar_tensor_tensor(
            p1n[:, mo, :], pm[:], smn[:, mo:mo + 1], p1n[:, mo, :],
            op0=ALU.mult, op1=ALU.add)
        nc.gpsimd.tensor_tensor(
            p1n[:, mo, :], pP0[:, mo, :], p1n[:, mo, :], op=ALU.add)
        if q == MOC - 1:
            nc.scalar.dma_start(outv[:, MOC * j:MOC * (j + 1), :],
                                p1n[:, MOC * j:MOC * (j + 1), :])

    mm_pass(uh2, "z3", epi2)


def build_nc():
    nc = bacc.Bacc(target_bir_lowering=False)
    xs0 = nc.declare_dram_parameter("xs0", [P, N], bf16, isOutput=False)
    xs1 = nc.declare_dram_parameter("xs1", [64, N], bf16, isOutput=False)
    adjd = nc.declare_dram_parameter("adjd", [N, SC], fp8, isOutput=False)
    adjb = nc.declare_dram_parameter("adjb", [NCHUNK * N, SC], fp8,
                                     isOutput=False)
    w = nc.declare_dram_parameter("wcat", [P, 3 * P], bf16, isOutput=False)
    bfull = nc.declare_dram_parameter("bfull", [P, FL], bf16, isOutput=False)
    out = nc.declare_dram_parameter("out", [N, FL], f32, isOutput=True)
    dumps = None
    if DEBUG_DUMPS:
        dumps = {
            "dmp_d": nc.declare_dram_parameter("dmp_d", [1, SC], f32,
                                               isOutput=True),
            "dmp_s": nc.declare_dram_parameter("dmp_s", [P, NMO], f32,
                                               isOutput=True),
            "dmp_ys": nc.declare_dram_parameter("dmp_ys", [P, NMO * FL],
                                                RHS_DT, isOutput=True),
            "dmp_p2": nc.declare_dram_parameter("dmp_p2", [P, NMO * FL],
                                                bf16, isOutput=True),
            "dmp_p1": nc.declare_dram_parameter("dmp_p1", [P, NMO * FL],
                                                f32, isOutput=True),
            "dmp_p0": nc.declare_dram_parameter("dmp_p0", [P, NMO * FL],
                                                bf16, isOutput=True),
            "dmp_m": nc.declare_dram_parameter("dmp_m", [P, NMO * FL],
                                               f32, isOutput=True),
            "dmp_uh2": nc.declare_dram_parameter("dmp_uh2", [P, NMO * FL],
                                                 RHS_DT, isOutput=True),
        }
    with tile.TileContext(nc) as tc, ExitStack() as ctx:
        _graph_kernel(ctx, tc, xs0, xs1, adjd, adjb, w, bfull, out, dumps)
    nc.compile()
    return nc


def make_in_maps(x, adj, weight, bias):
    wcat = np.zeros((P, 3 * P), np.float32)
    mats = [2.0 * weight[2], weight[1], weight[0] - weight[2]]
    for j, m in enumerate(mats):
        for s in range(4):
            wcat[32 * s:32 * (s + 1),
                 P * j + 32 * s:P * j + 32 * (s + 1)] = m
    wcat = wcat.astype(ml_dtypes.bfloat16)
    bfull = np.tile(np.asarray(bias, np.float32), (P, NPAIR)).astype(
        ml_dtypes.bfloat16)
    adj8 = np.ascontiguousarray(
        adj.reshape(N, NCHUNK, SC).transpose(1, 0, 2)).reshape(
            NCHUNK * N, SC).astype(ml_dtypes.float8_e4m3)
    in_maps = []
    for k in range(NCORES):
        b, t0 = k // 2, NPAIR * (k % 2)
        xk = x[b][:, :, t0:t0 + NPAIR].transpose(2, 0, 1)  # [pair, c, n]
        in_maps.append({
            "xs0": np.ascontiguousarray(xk[0:4]).reshape(P, N).astype(
                ml_dtypes.bfloat16),
            "xs1": np.ascontiguousarray(xk[4:6]).reshape(64, N).astype(
                ml_dtypes.bfloat16),
            "adjd": np.ascontiguousarray(
                adj[:, SC * k:SC * (k + 1)]).astype(ml_dtypes.float8_e4m3),
            "adjb": adj8,
            "wcat": wcat,
            "bfull": bfull,
        })
    return in_maps


def kernel(x, adj, weight, bias, _trace=False, _tmpdir=None):
    if "nc" not in _CACHE:
        _CACHE["nc"] = build_nc()
    nc = _CACHE["nc"]
    in_maps = make_in_maps(
        np.asarray(x, np.float32), np.asarray(adj, np.float32),
        np.asarray(weight, np.float32), np.asarray(bias, np.float32))
    res = run_bass_kernel_spmd(nc, in_maps, core_ids=list(range(NCORES)),
                               trace=_trace, tmpdir=_tmpdir)
    _CACHE["last_result"] = res
    full = np.empty((B, C, N, T), np.float32)
    for k, r in enumerate(res.results):
        b, t0 = k // 2, NPAIR * (k % 2)
        part = r["out"].reshape(N, NPAIR, C)          # [n, pair, c]
        full[b, :, :, t0:t0 + NPAIR] = part.transpose(2, 0, 1)
    return full
